# revision 15
# baseline (speedup 1.0000x reference)
"""GAT 2-layer node classification on 8 Trainium2 NeuronCores.

Strategy (self-contained; shapes hardcoded for the fixed problem):
  - Host: add self-loops, sort edges by dst, shard dst nodes contiguously
    across 8 cores, build per-core gather index arrays (int16, split-table
    trick for N>32767) and one-hot-generation metadata. Host does ONLY
    index manipulation + table assembly; all FP compute runs on device.
  - D1 (device): h1|a1 = x @ [W1 | W1*att-blockdiag]  (node-sharded dense)
  - host: assemble gather table T1 rows = [h1+b1, a_s1] (+dummy rows)
  - D2 (device): layer-1 GAT aggregation per dst tile via dma_gather +
    one-hot matmul segment softmax/sum, relu, on-core dense layer 2
  - host: assemble T2
  - D3 (device): layer-2 aggregation, leaky_relu, classifier
  - host: transpose/concat final logits.
"""

import os
import sys

import numpy as np

sys.path.insert(0, "/opt/trn_rl_repo")

import concourse.bass as bass  # noqa: E402
import concourse.mybir as mybir  # noqa: E402
import concourse.tile as tile  # noqa: E402
from concourse import bacc  # noqa: E402
from concourse.bass_utils import run_bass_kernel_spmd  # noqa: E402

# ---------------------------------------------------------------- constants
N = 50000
FIN = 256
H1, C1 = 4, 64
D1 = H1 * C1  # 256
H2, C2 = 4, 32
D2 = H2 * C2  # 128
NCLS = 7
NCORES = 8
P = 128
SHARD = 6272  # 49 tiles of 128 (>= ceil(50000/8))
NTILES = SHARD // P  # 49
NPAD = SHARD * NCORES  # 50176

LOWN = 32767  # nodes with src index in low table zone (0..32766)
TROWS = 50002  # 1 dummy + 32767 low + 1 dummy + 17233 high
HIGH_BASE = 32768  # table row of high-zone dummy

NEG_ATT = 0.2
NEG_ACT = 0.01
DUMMY_AS = -200.0

USE_BF16 = os.environ.get("GAT_DTYPE", "f32") == "bf16"
TRACE = os.environ.get("GAT_TRACE", "0") == "1"
MAXG = int(os.environ.get("GAT_MAXG", "4"))  # max 128-chunks per dma_gather
DDS = int(os.environ.get("GAT_DDS", "16384"))  # SWDGE scratch ring bytes
HOTBUFS = int(os.environ.get("GAT_BUFS", "2"))  # hot pool depth

if USE_BF16:
    import ml_dtypes

    NPDT = ml_dtypes.bfloat16
    DT = mybir.dt.bfloat16
    # table row layout in DT units; a_s stored as raw f32 (2 bf16 slots each)
    ELEM1 = 384  # [h 0:256 | a_s f32 @ slots 256:264 | pad]
    AS1_F32OFF = 128  # f32-element offset of a_s within a bitcast row
    ROWF1 = 192  # f32 elements per row
    ELEM2 = 256  # [h2 0:128 | a_s f32 @ slots 128:136 | pad]
    AS2_F32OFF = 64
    ROWF2 = 128
else:
    NPDT = np.float32
    DT = mybir.dt.float32
    ELEM1 = 320  # [h 0:256 | a_s 256:260 | pad]
    AS1_F32OFF = 256
    ROWF1 = 320
    ELEM2 = 192  # [h2 0:128 | a_s 128:132 | pad]
    AS2_F32OFF = 128
    ROWF2 = 192

F32 = mybir.dt.float32
F32R = mybir.dt.float32r


def _mm(ap):
    """matmul operand view: reinterpret f32 as float32r (replicated fp32,
    1 cycle/row on PE when out free >= 256, vs 4 for plain fp32)."""
    if ap.dtype == F32:
        return ap.bitcast(F32R)
    return ap


def row_of_node(n):
    """table row for node index array n (vectorized)."""
    return np.where(n < LOWN, n + 1, n + 2)


# ---------------------------------------------------------------- host plan
class Plan:
    pass


def build_plan(edge_index):
    src = np.asarray(edge_index[0], dtype=np.int64)
    dst = np.asarray(edge_index[1], dtype=np.int64)
    loops = np.arange(N, dtype=np.int64)
    src = np.concatenate([src, loops])
    dst = np.concatenate([dst, loops])
    order = np.argsort(dst, kind="stable")
    src = src[order].astype(np.int32)
    dst = dst[order].astype(np.int32)
    starts = np.searchsorted(dst, np.arange(N + 1))

    # per (core, tile): low/high slot lists
    low_lists = [[None] * NTILES for _ in range(NCORES)]
    high_lists = [[None] * NTILES for _ in range(NCORES)]
    for c in range(NCORES):
        for t in range(NTILES):
            g0 = c * SHARD + t * P
            g1 = min(g0 + P, N)
            if g1 > g0:
                e0, e1 = starts[g0], starts[g1]
                s = src[e0:e1]
                dl = (dst[e0:e1] - g0).astype(np.int32)
                m = s < LOWN
                lo_idx = (s[m] + 1).astype(np.int32)
                lo_dl = dl[m]
                hi_idx = (s[~m] - LOWN + 1).astype(np.int32)
                hi_dl = dl[~m]
            else:
                lo_idx = np.zeros(0, np.int32)
                lo_dl = np.zeros(0, np.int32)
                hi_idx = np.zeros(0, np.int32)
                hi_dl = np.zeros(0, np.int32)
            # pad nodes (>= N) in this tile get one dummy low slot each so
            # their softmax denominator is finite (no NaN in discarded rows)
            npad_nodes = (g0 + P) - max(g1, g0)
            if npad_nodes > 0:
                padl = np.arange(P - npad_nodes, P, dtype=np.int32)
                lo_idx = np.concatenate([lo_idx, np.zeros(npad_nodes, np.int32)])
                lo_dl = np.concatenate([lo_dl, padl])
            low_lists[c][t] = (lo_idx, lo_dl)
            high_lists[c][t] = (hi_idx, hi_dl)

    CL = [max(-(-len(low_lists[c][t][0]) // P) for c in range(NCORES)) for t in range(NTILES)]
    CH = [max(-(-len(high_lists[c][t][0]) // P) for c in range(NCORES)) for t in range(NTILES)]
    CL = [max(v, 1) for v in CL]
    C = [CL[t] + CH[t] for t in range(NTILES)]
    CMAX = max(C)
    CUM = np.concatenate([[0], np.cumsum(C)]).astype(np.int64)
    TOTC = int(CUM[-1])
    COLS = TOTC * 8  # int16 index columns

    # build per-core arrays
    idx16 = np.zeros((NCORES, 128, COLS), np.int16)
    dstslot = np.full((NCORES, 128, TOTC), 127.0, np.float32)
    dstrow = np.full((NCORES, NTILES, CMAX * P), 127.0, np.float32)

    for c in range(NCORES):
        col = 0
        for t in range(NTILES):
            for (lst, nchunk) in ((low_lists[c][t], CL[t]), (high_lists[c][t], CH[t])):
                idx, dl = lst
                nslot = nchunk * P
                vi = np.zeros(nslot, np.int16)
                vi[: len(idx)] = idx.astype(np.int16)
                vd = np.full(nslot, 127.0, np.float32)
                vd[: len(dl)] = dl.astype(np.float32)
                if nchunk > 0:
                    # idx16 wrapped layout: slot i -> [i%16, col + i//16]
                    seg = vi.reshape(-1, 16).T  # [16, nslot/16]
                    for rep in range(8):
                        idx16[c, rep * 16 : rep * 16 + 16, col : col + nslot // 16] = seg
                    # dstslot: [e, chunkcol] = dl of slot chunk*128+e
                    cbase = np.searchsorted(CUM, 0)  # placeholder
                    col += nslot // 16
                # record dstslot/dstrow below using tile-relative positions
            # fill dstslot/dstrow for this tile
            cbase = CUM[t]
            nslot_t = C[t] * P
            vd_all = np.full(nslot_t, 127.0, np.float32)
            lo_idx, lo_dl = low_lists[c][t]
            hi_idx, hi_dl = high_lists[c][t]
            vd_all[: len(lo_dl)] = lo_dl
            hbase = CL[t] * P
            vd_all[hbase : hbase + len(hi_dl)] = hi_dl
            dstslot[c, :, cbase : cbase + C[t]] = vd_all.reshape(C[t], P).T
            dstrow[c, t, : nslot_t] = vd_all

    # gather segment offsets (in idx16 columns), per tile: (lo_off, hi_off)
    seg_off = []
    col = 0
    for t in range(NTILES):
        lo = col
        col += CL[t] * 8
        hi = col
        col += CH[t] * 8
        seg_off.append((lo, hi))
    assert col == COLS

    pl = Plan()
    pl.src, pl.dst = src, dst
    pl.CL, pl.CH, pl.C, pl.CMAX, pl.CUM = CL, CH, C, CMAX, CUM
    pl.TOTC, pl.COLS, pl.seg_off = TOTC, COLS, seg_off
    pl.idx16 = idx16
    pl.dstslot = dstslot.astype(NPDT)
    import ml_dtypes as _mld
    pl.dstrow = dstrow.astype(_mld.bfloat16)
    return pl


# ------------------------------------------------------------ device builds
def build_d1(elem_out):
    """dense: out[6272, 264] = xT_shard.T @ [W | Wa] + [b | 0]"""
    nc = bacc.Bacc("TRN2", target_bir_lowering=False, debug=False, num_devices=NCORES)
    xT = nc.dram_tensor("xT", [FIN, SHARD], DT, kind="ExternalInput")
    wcat = nc.dram_tensor("wcat", [P, 2 * elem_out], DT, kind="ExternalInput")
    brep = nc.dram_tensor("brep", [P, elem_out], F32, kind="ExternalInput")
    out = nc.dram_tensor("out", [SHARD, elem_out], F32, kind="ExternalOutput")

    with tile.TileContext(nc) as tc:
        with (
            tc.tile_pool(name="consts", bufs=1) as cpool,
            tc.tile_pool(name="lhs", bufs=3) as lpool,
            tc.tile_pool(name="res", bufs=3) as rpool,
            tc.tile_pool(name="ps", bufs=2, space="PSUM") as ppool,
        ):
            w_sb = cpool.tile([P, 2 * elem_out], DT)
            nc.sync.dma_start(w_sb, wcat.ap())
            b_sb = cpool.tile([P, elem_out], F32)
            nc.sync.dma_start(b_sb, brep.ap())
            for t in range(NTILES):
                xt0 = lpool.tile([P, P], DT, tag="xt0")
                nc.sync.dma_start(xt0, xT.ap()[0:128, t * P : (t + 1) * P])
                xt1 = lpool.tile([P, P], DT, tag="xt1")
                nc.sync.dma_start(xt1, xT.ap()[128:256, t * P : (t + 1) * P])
                ps = ppool.tile([P, elem_out], F32, space="PSUM")
                nc.tensor.matmul(ps, lhsT=xt0, rhs=w_sb[:, 0:elem_out], start=True, stop=False)
                nc.tensor.matmul(ps, lhsT=xt1, rhs=w_sb[:, elem_out:], start=False, stop=True)
                res = rpool.tile([P, elem_out], F32)
                nc.vector.tensor_tensor(out=res, in0=ps, in1=b_sb, op=mybir.AluOpType.add)
                nc.sync.dma_start(out.ap()[t * P : (t + 1) * P, :], res)
    nc.compile()
    return nc


def build_agg(pl, layer):
    """Aggregation dispatch. layer=1: gather T1, produce T2 rows (h2|a2).
    layer=2: gather T2, produce classifier logits [8, 6272]."""
    if layer == 1:
        ELEM, ASOFF, ROWF = ELEM1, AS1_F32OFF, ROWF1
        DFEAT, NH, CH_ = D1, H1, C1  # 256, 4, 64
        ELEM_OUT2 = D2 + 8  # 136 dense-2 output row
    else:
        ELEM, ASOFF, ROWF = ELEM2, AS2_F32OFF, ROWF2
        DFEAT, NH, CH_ = D2, H2, C2  # 128, 4, 32

    NFH = DFEAT // P  # feature partition-tiles (2 for L1, 1 for L2)
    RHS_W = DFEAT + 4  # matmul rhs width: feats + p
    USE_R = not USE_BF16  # fp32r on the big feat matmul
    if USE_R:
        RHS_W = max(RHS_W, 256)  # fp32r needs out free >= 256 for 1cyc/row
    GPAD = 0

    nc = bacc.Bacc("TRN2", target_bir_lowering=False, debug=False, num_devices=NCORES,
                   dynamic_dma_scratch_size=DDS)
    T = nc.dram_tensor("T", [TROWS, ELEM], DT, kind="ExternalInput")
    idx_d = nc.dram_tensor("idx", [128, pl.COLS], mybir.dt.int16, kind="ExternalInput")
    dstslot_d = nc.dram_tensor("dstslot", [128, pl.TOTC], DT, kind="ExternalInput")
    dstrow_d = nc.dram_tensor("dstrow", [NTILES, pl.CMAX * P], mybir.dt.bfloat16, kind="ExternalInput")
    ad_d = nc.dram_tensor("ad", [P, NTILES * 4], F32, kind="ExternalInput")
    iota_row_d = nc.dram_tensor("iota_row", [P, P], DT, kind="ExternalInput")
    iota_col_d = nc.dram_tensor("iota_col", [P, 1], F32, kind="ExternalInput")
    ones_d = nc.dram_tensor("ones1", [1, P], mybir.dt.bfloat16, kind="ExternalInput")
    if layer == 1:
        W2W = ELEM_OUT2 if USE_BF16 else 256
        w2cat_d = nc.dram_tensor("w2cat", [P, 2 * W2W], DT, kind="ExternalInput")
        b2rep_d = nc.dram_tensor("b2rep", [P, ELEM_OUT2], F32, kind="ExternalInput")
        ident_d = nc.dram_tensor("ident", [P, P], DT, kind="ExternalInput")
        out = nc.dram_tensor("out", [SHARD, ELEM_OUT2], F32, kind="ExternalOutput")
    else:
        wl_d = nc.dram_tensor("wl", [P, 8], DT, kind="ExternalInput")
        bl_d = nc.dram_tensor("bl", [8, 1], F32, kind="ExternalInput")
        ident_d = nc.dram_tensor("ident", [P, P], DT, kind="ExternalInput")
        out = nc.dram_tensor("out", [8, SHARD], F32, kind="ExternalOutput")

    T_lo = T.ap()[0:HIGH_BASE, :]
    T_hi = T.ap()[HIGH_BASE:TROWS, :]

    with tile.TileContext(nc) as tc:
        with (
            tc.tile_pool(name="consts", bufs=1) as cpool,
            tc.tile_pool(name="gather", bufs=HOTBUFS) as gpool,
            tc.tile_pool(name="onehot", bufs=HOTBUFS) as opool,
            tc.tile_pool(name="scores", bufs=HOTBUFS) as spool,
            tc.tile_pool(name="small", bufs=3) as smpool,
            tc.tile_pool(name="drow", bufs=3) as drpool,
            tc.tile_pool(name="psA", bufs=2, space="PSUM") as psA,   # dstrow bcast
            tc.tile_pool(name="psB", bufs=2, space="PSUM") as psB,   # ad scores
            tc.tile_pool(name="psC", bufs=2, space="PSUM") as psC,   # feat accum
            tc.tile_pool(name="psD", bufs=1, space="PSUM") as psD,   # transpose
            tc.tile_pool(name="psE", bufs=1, space="PSUM") as psE,   # dense2/cls
        ):
            # ---- constants / global loads
            idx_sb = cpool.tile([128, pl.COLS], mybir.dt.int16)
            nc.sync.dma_start(idx_sb, idx_d.ap())
            dstslot_sb = cpool.tile([128, pl.TOTC], DT)
            nc.sync.dma_start(dstslot_sb, dstslot_d.ap())
            ad_sb = cpool.tile([P, NTILES * 4], F32)
            nc.sync.dma_start(ad_sb, ad_d.ap())
            iota_row = cpool.tile([P, P], DT)
            nc.sync.dma_start(iota_row, iota_row_d.ap())
            iota_col = cpool.tile([P, 1], F32)
            nc.sync.dma_start(iota_col, iota_col_d.ap())
            ones1 = cpool.tile([1, P], mybir.dt.bfloat16)
            nc.sync.dma_start(ones1, ones_d.ap())
            ident = cpool.tile([P, P], DT)
            nc.sync.dma_start(ident, ident_d.ap())
            if layer == 1:
                w2_sb = cpool.tile([P, 2 * W2W], DT)
                nc.sync.dma_start(w2_sb, w2cat_d.ap())
                if not USE_BF16:
                    w2_r = cpool.tile([P, 2 * W2W], F32)
                    nc.vector.tensor_copy(out=w2_r.bitcast(F32R), in_=w2_sb)
                else:
                    w2_r = w2_sb
                b2_sb = cpool.tile([P, ELEM_OUT2], F32)
                nc.sync.dma_start(b2_sb, b2rep_d.ap())
            else:
                wl_sb = cpool.tile([P, 8], DT)
                nc.sync.dma_start(wl_sb, wl_d.ap())
                bl_sb = cpool.tile([8, 1], F32)
                nc.sync.dma_start(bl_sb, bl_d.ap())
                outbuf = cpool.tile([8, SHARD], F32)

            for t in range(NTILES):
                C = pl.C[t]
                CL, CH = pl.CL[t], pl.CH[t]
                lo_off, hi_off = pl.seg_off[t]

                # ---- gather rows for this tile's edge slots
                G = gpool.tile([128, pl.CMAX * ELEM + GPAD], DT, tag="G", name="G")[:, : C * ELEM]
                G3 = G.rearrange("p (c e) -> p c e", e=ELEM)
                for (nch, cb, off, src_ap) in (
                    (CL, 0, lo_off, T_lo),
                    (CH, CL, hi_off, T_hi),
                ):
                    for p0 in range(0, nch, MAXG):
                        pc = min(MAXG, nch - p0)
                        nc.gpsimd.dma_gather(
                            G3[:, cb + p0 : cb + p0 + pc, :],
                            src_ap,
                            idx_sb[:, off + p0 * 8 : off + (p0 + pc) * 8],
                            pc * P,
                            pc * P,
                            ELEM,
                        )

                # ---- PT one-hot [d, C*128]: via PE broadcast of dstrow + is_equal
                drow = drpool.tile([1, pl.CMAX * P], mybir.dt.bfloat16, tag="drow", name="drow")[:, : C * P]
                nc.sync.dma_start(drow, dstrow_d.ap()[t : t + 1, 0 : C * P])
                PT = opool.tile([128, pl.CMAX * P], F32, tag="PT", name="PT")[:, : C * P]
                for s0 in range(0, C * P, 512):
                    seg = min(512, C * P - s0)
                    psd = psA.tile([P, 512], F32, tag="psd", space="PSUM")
                    nc.tensor.matmul(
                        psd[:, 0:seg], lhsT=ones1, rhs=drow[:, s0 : s0 + seg],
                        start=True, stop=True,
                    )
                    dsb = drpool.tile([P, 512], F32, tag="dsb", name="dsb")
                    nc.scalar.copy(out=dsb[:, 0:seg], in_=psd[:, 0:seg])
                    nc.vector.tensor_scalar(
                        out=PT[:, s0 : s0 + seg], in0=dsb[:, 0:seg],
                        scalar1=iota_col, scalar2=None,
                        op0=mybir.AluOpType.is_equal,
                    )

                # ---- P one-hot [e, C*128] (DT)
                Pm = opool.tile([128, pl.CMAX * P], DT, tag="Pm", name="Pm")[:, : C * P]
                if USE_R:
                    Pm = Pm.bitcast(F32R)
                Pm3 = Pm.rearrange("p (c d) -> p c d", d=P)
                nc.vector.tensor_tensor(
                    out=Pm3,
                    in0=dstslot_sb[:, pl.CUM[t] : pl.CUM[t] + C].to_broadcast([128, C, P]),
                    in1=iota_row.unsqueeze(1).to_broadcast([128, C, P]),
                    op=mybir.AluOpType.is_equal,
                )

                # ---- a_d per edge: psum_sc[e, c*4+h] = sum_d PT[d, e] * a_d[d, h]
                pssc = psB.tile([P, pl.CMAX * 4], F32, tag="pssc", space="PSUM")
                for c in range(C):
                    nc.tensor.matmul(
                        pssc[:, c * 4 : (c + 1) * 4],
                        lhsT=PT[:, c * P : (c + 1) * P],
                        rhs=ad_sb[:, t * 4 : (t + 1) * 4],
                        start=True, stop=True,
                    )

                # ---- scores: e = leaky(a_s + a_d); p = exp(e)
                G_f = G.bitcast(F32)
                G_f3 = G_f.rearrange("p (c e) -> p c e", e=ROWF)
                as_view = G_f3[:, :, ASOFF : ASOFF + 4]
                esum = spool.tile([128, pl.CMAX * 4], F32, tag="esum", name="esum")[:, : C * 4]
                esum3 = esum.rearrange("p (c h) -> p c h", h=4)
                nc.vector.tensor_tensor(
                    out=esum3, in0=as_view,
                    in1=pssc[:, : C * 4].rearrange("p (c h) -> p c h", h=4),
                    op=mybir.AluOpType.add,
                )
                nc.vector.scalar_tensor_tensor(
                    out=esum, in0=esum, scalar=NEG_ATT, in1=esum,
                    op0=mybir.AluOpType.mult, op1=mybir.AluOpType.max,
                )
                # p -> written into the a_s slots (consumed above) to form
                # a contiguous matmul rhs [feats | p] per chunk
                RA = spool.tile([128, pl.CMAX * RHS_W], DT, tag="RA", name="RA")[:, : C * RHS_W]
                if USE_R:
                    RA = RA.bitcast(F32R)
                RA3 = RA.rearrange("p (c e) -> p c e", e=RHS_W)
                feat4 = G3[:, :, 0:DFEAT].rearrange("p c (h f) -> p c h f", f=CH_)
                feat4o = RA3[:, :, 0:DFEAT].rearrange("p c (h f) -> p c h f", f=CH_)
                esum4 = esum.rearrange("p (c h) -> p c h", h=4)
                if USE_BF16:
                    # exp pre-expanded by ACT (redundant transcendentals are
                    # cheaper than a DVE slow-mode broadcast multiply): the
                    # all-contiguous bf16 multiply then runs in DVE 2x mode.
                    pexp = spool.tile([128, pl.CMAX * DFEAT], DT, tag="pexp", name="pexp")[:, : C * DFEAT]
                    pexp4 = pexp.rearrange("p (c h f) -> p c h f", h=4, f=CH_)
                    nc.scalar.activation(
                        out=pexp4,
                        in_=esum4.to_broadcast([128, C, 4, CH_]),
                        func=mybir.ActivationFunctionType.Exp,
                    )
                    nc.vector.tensor_copy(
                        out=RA3[:, :, DFEAT : DFEAT + 4], in_=pexp4[:, :, :, 0:1].rearrange("p c h f -> p c (h f)")
                    )
                    nc.vector.tensor_tensor(out=feat4o, in0=feat4, in1=pexp4, op=mybir.AluOpType.mult)
                else:
                    p_sb = spool.tile([128, pl.CMAX * 4], F32, tag="p_sb", name="p_sb")[:, : C * 4]
                    p_sb3 = p_sb.rearrange("p (c h) -> p c h", h=4)
                    nc.scalar.activation(
                        out=p_sb3,
                        in_=esum4,
                        func=mybir.ActivationFunctionType.Exp,
                    )
                    # p into the rhs tail columns (cast/round on write)
                    nc.vector.tensor_copy(out=RA3[:, :, DFEAT : DFEAT + 4], in_=p_sb3)
                    pb = p_sb3.to_broadcast([128, C, 4, CH_])
                    nc.vector.tensor_tensor(out=feat4o, in0=feat4, in1=pb, op=mybir.AluOpType.mult)

                # ---- accumulate: out[d, 0:DFEAT]=feats, [DFEAT:DFEAT+4]=denom
                psout = psC.tile([P, RHS_W], F32, tag="psout", space="PSUM")
                for c in range(C):
                    nc.tensor.matmul(
                        psout,
                        lhsT=Pm[:, c * P : (c + 1) * P],
                        rhs=RA[:, c * RHS_W : (c + 1) * RHS_W],
                        start=(c == 0), stop=(c == C - 1),
                    )

                # ---- normalize
                recip = smpool.tile([P, 4], F32, tag="recip")
                nc.vector.reciprocal(recip, psout[:, DFEAT : DFEAT + 4])
                o1 = smpool.tile([P, DFEAT], F32, tag="o1")
                nc.vector.tensor_tensor(
                    out=o1.rearrange("p (h f) -> p h f", f=CH_),
                    in0=psout[:, 0:DFEAT].rearrange("p (h f) -> p h f", f=CH_),
                    in1=recip.to_broadcast([P, 4, CH_]),
                    op=mybir.AluOpType.mult,
                )

                if layer == 1:
                    # relu -> r1 (DT), transpose, dense-2, +b2, write T2 rows
                    r1 = smpool.tile([P, DFEAT], DT, tag="r1")
                    nc.vector.tensor_scalar(
                        out=r1, in0=o1, scalar1=0.0, scalar2=None,
                        op0=mybir.AluOpType.max,
                    )
                    pse = psE.tile([P, W2W], F32, tag="pse", space="PSUM")
                    for h in range(NFH):
                        pst = psD.tile([P, P], DT, tag="pst", space="PSUM")
                        nc.tensor.transpose(pst, r1[:, h * P : (h + 1) * P], ident)
                        r1T = smpool.tile([P, P], DT, tag="r1T")
                        r1To = r1T.bitcast(F32R) if not USE_BF16 else r1T
                        nc.scalar.copy(out=r1To, in_=pst)
                        nc.tensor.matmul(
                            pse,
                            lhsT=r1To,
                            rhs=w2_r.bitcast(F32R)[:, h * W2W : h * W2W + W2W]
                            if not USE_BF16 else w2_r[:, h * W2W : h * W2W + W2W],
                            start=(h == 0), stop=(h == NFH - 1),
                        )
                    t2row = smpool.tile([P, ELEM_OUT2], F32, tag="t2row")
                    nc.vector.tensor_tensor(out=t2row, in0=pse[:, 0:ELEM_OUT2], in1=b2_sb, op=mybir.AluOpType.add)
                    nc.sync.dma_start(out.ap()[t * P : (t + 1) * P, :], t2row)
                else:
                    # leaky(0.01) -> transpose -> classifier -> outbuf
                    o2 = smpool.tile([P, DFEAT], DT, tag="o2")
                    nc.vector.scalar_tensor_tensor(
                        out=o2, in0=o1, scalar=NEG_ACT, in1=o1,
                        op0=mybir.AluOpType.mult, op1=mybir.AluOpType.max,
                    )
                    pst = psD.tile([P, P], DT, tag="pst", space="PSUM")
                    nc.tensor.transpose(pst, o2, ident)
                    o2T = smpool.tile([P, P], DT, tag="o2T")
                    nc.scalar.copy(out=o2T, in_=pst)
                    psc = psE.tile([8, P], F32, tag="psc", space="PSUM")
                    nc.tensor.matmul(psc, lhsT=wl_sb, rhs=o2T, start=True, stop=True)
                    nc.vector.tensor_scalar(
                        out=outbuf[:, t * P : (t + 1) * P], in0=psc,
                        scalar1=bl_sb, scalar2=None, op0=mybir.AluOpType.add,
                    )
            if layer == 2:
                nc.sync.dma_start(out.ap(), outbuf)
    nc.compile()
    return nc


# ------------------------------------------------------------------ helpers
def _wcat(W, att_src, att_dst, heads, chan):
    """[W | W@blockdiag(att_src) | W@blockdiag(att_dst)] -> [K, D+8]"""
    K, Dh = W.shape
    wa_s = np.zeros((K, heads), np.float32)
    wa_d = np.zeros((K, heads), np.float32)
    for h in range(heads):
        wa_s[:, h] = W[:, h * chan : (h + 1) * chan] @ att_src[h]
        wa_d[:, h] = W[:, h * chan : (h + 1) * chan] @ att_dst[h]
    return np.concatenate([W, wa_s, wa_d], axis=1).astype(np.float32)


def _chunk_major(Wfull, width=None):
    """[256, E] -> [128, 2*width] (K-chunk-major for SBUF, zero-padded)"""
    e = Wfull.shape[1]
    width = width or e
    out = np.zeros((128, 2 * width), Wfull.dtype)
    out[:, 0:e] = Wfull[0:128, :]
    out[:, width : width + e] = Wfull[128:256, :]
    return out


def _make_table(h_plus_b, a_s, rowf, elem, asoff):
    """Assemble gather table [TROWS, elem] in DT with a_s stored as f32."""
    dfeat = h_plus_b.shape[1]
    Tf = np.zeros((TROWS, rowf), np.float32)
    if USE_BF16:
        Tb = np.zeros((TROWS, elem), NPDT)
        rows = row_of_node(np.arange(N))
        Tb[rows, 0:dfeat] = h_plus_b.astype(NPDT)
        Tf_view = Tb.view(np.uint8).reshape(TROWS, elem * 2)
        asf = np.zeros((TROWS, 4), np.float32)
        asf[rows] = a_s
        asf[0] = DUMMY_AS
        asf[HIGH_BASE] = DUMMY_AS
        Tf_view[:, asoff * 4 : asoff * 4 + 16] = asf.view(np.uint8).reshape(TROWS, 16)
        return Tb
    else:
        rows = row_of_node(np.arange(N))
        Tf[rows, 0:dfeat] = h_plus_b
        Tf[:, asoff : asoff + 4] = DUMMY_AS
        Tf[rows, asoff : asoff + 4] = a_s
        return Tf


def _ad_input(a_d):
    """[NPAD, 4] padded a_d -> per-core [128, NTILES*4]"""
    out = np.zeros((NCORES, P, NTILES * 4), np.float32)
    for c in range(NCORES):
        blk = a_d[c * SHARD : (c + 1) * SHARD].reshape(NTILES, P, 4)
        out[c] = blk.transpose(1, 0, 2).reshape(P, NTILES * 4)
    return out


_CACHE = {}


def _run(nc, in_maps, tag):
    trace = TRACE
    if trace:
        try:
            from antenv.axon_hooks import get_axon_ntff_profile_hook  # noqa: F401
        except ImportError:
            trace = False
    res = run_bass_kernel_spmd(nc, in_maps, core_ids=list(range(NCORES)), trace=trace)
    if trace and res.exec_time_ns:
        print(f"[{tag}] exec_time_ns = {res.exec_time_ns}", file=sys.stderr)
        _CACHE.setdefault("times", {})[tag] = res.exec_time_ns
    return res.results


# -------------------------------------------------------------------- main
def kernel(x, edge_index, W1, att_src1, att_dst1, b1, W2, att_src2, att_dst2, b2, Wl, bl):
    x = np.asarray(x, np.float32)
    W1 = np.asarray(W1, np.float32)
    W2 = np.asarray(W2, np.float32)
    Wl = np.asarray(Wl, np.float32)
    b1 = np.asarray(b1, np.float32)
    b2 = np.asarray(b2, np.float32)
    bl = np.asarray(bl, np.float32)
    att_src1 = np.asarray(att_src1, np.float32)
    att_dst1 = np.asarray(att_dst1, np.float32)
    att_src2 = np.asarray(att_src2, np.float32)
    att_dst2 = np.asarray(att_dst2, np.float32)

    pl = build_plan(np.asarray(edge_index))

    iota_row = np.tile(np.arange(P, dtype=np.float32)[None, :], (P, 1)).astype(NPDT)
    iota_col = np.arange(P, dtype=np.float32).reshape(P, 1)
    import ml_dtypes as _mld
    ones1 = np.ones((1, P), _mld.bfloat16)
    ident = np.eye(P, dtype=np.float32).astype(NPDT)

    # ---------------- D1: dense layer-1
    w1cat = _wcat(W1, att_src1, att_dst1, H1, C1)  # [256, 264]
    ELEM_D1 = D1 + 8
    xT = np.zeros((FIN, NPAD), np.float32)
    xT[:, 0:N] = x.T
    d1_in = []
    for c in range(NCORES):
        d1_in.append({
            "xT": xT[:, c * SHARD : (c + 1) * SHARD].astype(NPDT),
            "wcat": _chunk_major(w1cat).astype(NPDT),
            "brep": np.tile(np.concatenate([b1, np.zeros(8, np.float32)])[None, :], (P, 1)),
        })
    if "d1" not in _CACHE:
        _CACHE["d1"] = build_d1(ELEM_D1)
    r1 = _run(_CACHE["d1"], d1_in, "d1")
    ha1 = np.concatenate([r["out"] for r in r1], axis=0)[0:N]  # [N, 264] = [h1+b1 | a_s | a_d]

    # ---------------- host: assemble T1 + a_d input
    T1 = _make_table(ha1[:, 0:D1], ha1[:, D1 : D1 + 4], ROWF1, ELEM1, AS1_F32OFF)
    ad1 = np.zeros((NPAD, 4), np.float32)
    ad1[0:N] = ha1[:, D1 + 4 : D1 + 8]
    ad1_in = _ad_input(ad1)

    # ---------------- D2: layer-1 aggregation + dense layer-2
    w2cat = _wcat(W2, att_src2, att_dst2, H2, C2)  # [256, 136]
    ELEM_OUT2 = D2 + 8
    d2_in = []
    for c in range(NCORES):
        d2_in.append({
            "T": T1,
            "idx": pl.idx16[c],
            "dstslot": pl.dstslot[c],
            "dstrow": pl.dstrow[c],
            "ad": ad1_in[c],
            "iota_row": iota_row,
            "iota_col": iota_col,
            "ones1": ones1,
            "ident": ident,
            "w2cat": _chunk_major(w2cat, ELEM_OUT2 if USE_BF16 else 256).astype(NPDT),
            "b2rep": np.tile(np.concatenate([b2, np.zeros(8, np.float32)])[None, :], (P, 1)),
        })
    key = ("d2", pl.COLS, pl.TOTC, tuple(pl.C))
    if key not in _CACHE:
        _CACHE[key] = build_agg(pl, 1)
    r2 = _run(_CACHE[key], d2_in, "d2")
    ha2 = np.concatenate([r["out"] for r in r2], axis=0)[0:NPAD]  # [NPAD, 136]
    ha2n = np.zeros((N, ELEM_OUT2), np.float32)
    ha2n[:, :] = ha2[0:N]

    # ---------------- host: assemble T2 + a_d input
    T2 = _make_table(ha2n[:, 0:D2], ha2n[:, D2 : D2 + 4], ROWF2, ELEM2, AS2_F32OFF)
    ad2 = np.zeros((NPAD, 4), np.float32)
    ad2[0:N] = ha2n[:, D2 + 4 : D2 + 8]
    ad2_in = _ad_input(ad2)

    # ---------------- D3: layer-2 aggregation + classifier
    wl8 = np.zeros((P, 8), np.float32)
    wl8[:, 0:NCLS] = Wl
    bl8 = np.zeros((8, 1), np.float32)
    bl8[0:NCLS, 0] = bl
    d3_in = []
    for c in range(NCORES):
        d3_in.append({
            "T": T2,
            "idx": pl.idx16[c],
            "dstslot": pl.dstslot[c],
            "dstrow": pl.dstrow[c],
            "ad": ad2_in[c],
            "iota_row": iota_row,
            "iota_col": iota_col,
            "ones1": ones1,
            "ident": ident,
            "wl": wl8.astype(NPDT),
            "bl": bl8,
        })
    key3 = ("d3", pl.COLS, pl.TOTC, tuple(pl.C))
    if key3 not in _CACHE:
        _CACHE[key3] = build_agg(pl, 2)
    r3 = _run(_CACHE[key3], d3_in, "d3")

    out = np.zeros((N, NCLS), np.float32)
    for c in range(NCORES):
        blk = r3[c]["out"]  # [8, SHARD]
        g0, g1 = c * SHARD, min((c + 1) * SHARD, N)
        if g1 > g0:
            out[g0:g1] = blk[0:NCLS, 0 : g1 - g0].T
    return out


# revision 19
# speedup vs baseline: 1.2992x; 1.2992x over previous
"""GAT 2-layer node classification on 8 Trainium2 NeuronCores.

Strategy (self-contained; shapes hardcoded for the fixed problem):
  - Host: add self-loops, sort edges by dst, shard dst nodes contiguously
    across 8 cores, build per-core gather index arrays (int16, split-table
    trick for N>32767) and one-hot-generation metadata. Host does ONLY
    index manipulation + table assembly; all FP compute runs on device.
  - D1 (device): h1|a1 = x @ [W1 | W1*att-blockdiag]  (node-sharded dense)
  - host: assemble gather table T1 rows = [h1+b1, a_s1] (+dummy rows)
  - D2 (device): layer-1 GAT aggregation per dst tile via dma_gather +
    one-hot matmul segment softmax/sum, relu, on-core dense layer 2
  - host: assemble T2
  - D3 (device): layer-2 aggregation, leaky_relu, classifier
  - host: transpose/concat final logits.
"""

import os
import sys

import numpy as np

sys.path.insert(0, "/opt/trn_rl_repo")

import concourse.bass as bass  # noqa: E402
import concourse.mybir as mybir  # noqa: E402
import concourse.tile as tile  # noqa: E402
from concourse import bacc  # noqa: E402
from concourse.bass_utils import run_bass_kernel_spmd  # noqa: E402

# ---------------------------------------------------------------- constants
N = 50000
FIN = 256
H1, C1 = 4, 64
D1 = H1 * C1  # 256
H2, C2 = 4, 32
D2 = H2 * C2  # 128
NCLS = 7
NCORES = 8
P = 128
SHARD = 6272  # 49 tiles of 128 (>= ceil(50000/8))
NTILES = SHARD // P  # 49
NPAD = SHARD * NCORES  # 50176

LOWN = 32767  # nodes with src index in low table zone (0..32766)
TROWS = 50002  # 1 dummy + 32767 low + 1 dummy + 17233 high
HIGH_BASE = 32768  # table row of high-zone dummy

NEG_ATT = 0.2
NEG_ACT = 0.01
DUMMY_AS = -200.0

USE_BF16 = os.environ.get("GAT_DTYPE", "f32") == "bf16"
TRACE = os.environ.get("GAT_TRACE", "0") == "1"
MAXG = int(os.environ.get("GAT_MAXG", "4"))  # max 128-chunks per dma_gather
DDS = int(os.environ.get("GAT_DDS", "16384"))  # SWDGE scratch ring bytes
HOTBUFS = int(os.environ.get("GAT_BUFS", "2"))  # hot pool depth

if USE_BF16:
    import ml_dtypes

    NPDT = ml_dtypes.bfloat16
    DT = mybir.dt.bfloat16
    # table row layout in DT units; a_s stored as raw f32 (2 bf16 slots each)
    ELEM1 = 384  # [h 0:256 | a_s f32 @ slots 256:264 | pad]
    AS1_F32OFF = 128  # f32-element offset of a_s within a bitcast row
    ROWF1 = 192  # f32 elements per row
    ELEM2 = 256  # [h2 0:128 | a_s f32 @ slots 128:136 | pad]
    AS2_F32OFF = 64
    ROWF2 = 128
else:
    NPDT = np.float32
    DT = mybir.dt.float32
    ELEM1 = 320  # [h 0:256 | a_s 256:260 | pad]
    AS1_F32OFF = 256
    ROWF1 = 320
    ELEM2 = 192  # [h2 0:128 | a_s 128:132 | pad]
    AS2_F32OFF = 128
    ROWF2 = 192

F32 = mybir.dt.float32
F32R = mybir.dt.float32r


def _mm(ap):
    """matmul operand view: reinterpret f32 as float32r (replicated fp32,
    1 cycle/row on PE when out free >= 256, vs 4 for plain fp32)."""
    if ap.dtype == F32:
        return ap.bitcast(F32R)
    return ap


def row_of_node(n):
    """table row for node index array n (vectorized)."""
    return np.where(n < LOWN, n + 1, n + 2)


# ---------------------------------------------------------------- host plan
class Plan:
    pass


def build_plan(edge_index):
    src = np.asarray(edge_index[0], dtype=np.int64)
    dst = np.asarray(edge_index[1], dtype=np.int64)
    loops = np.arange(N, dtype=np.int64)
    src = np.concatenate([src, loops])
    dst = np.concatenate([dst, loops])
    order = np.argsort(dst, kind="stable")
    src = src[order].astype(np.int32)
    dst = dst[order].astype(np.int32)
    starts = np.searchsorted(dst, np.arange(N + 1))

    # per (core, tile): low/high slot lists
    low_lists = [[None] * NTILES for _ in range(NCORES)]
    high_lists = [[None] * NTILES for _ in range(NCORES)]
    for c in range(NCORES):
        for t in range(NTILES):
            g0 = c * SHARD + t * P
            g1 = min(g0 + P, N)
            if g1 > g0:
                e0, e1 = starts[g0], starts[g1]
                s = src[e0:e1]
                dl = (dst[e0:e1] - g0).astype(np.int32)
                m = s < LOWN
                lo_idx = (s[m] + 1).astype(np.int32)
                lo_dl = dl[m]
                hi_idx = (s[~m] - LOWN + 1).astype(np.int32)
                hi_dl = dl[~m]
            else:
                lo_idx = np.zeros(0, np.int32)
                lo_dl = np.zeros(0, np.int32)
                hi_idx = np.zeros(0, np.int32)
                hi_dl = np.zeros(0, np.int32)
            # pad nodes (>= N) in this tile get one dummy low slot each so
            # their softmax denominator is finite (no NaN in discarded rows)
            npad_nodes = (g0 + P) - max(g1, g0)
            if npad_nodes > 0:
                padl = np.arange(P - npad_nodes, P, dtype=np.int32)
                lo_idx = np.concatenate([lo_idx, np.zeros(npad_nodes, np.int32)])
                lo_dl = np.concatenate([lo_dl, padl])
            low_lists[c][t] = (lo_idx, lo_dl)
            high_lists[c][t] = (hi_idx, hi_dl)

    CL = [max(-(-len(low_lists[c][t][0]) // P) for c in range(NCORES)) for t in range(NTILES)]
    CH = [max(-(-len(high_lists[c][t][0]) // P) for c in range(NCORES)) for t in range(NTILES)]
    CL = [max(v, 1) for v in CL]
    C = [CL[t] + CH[t] for t in range(NTILES)]
    CMAX = max(C)
    CUM = np.concatenate([[0], np.cumsum(C)]).astype(np.int64)
    TOTC = int(CUM[-1])
    COLS = TOTC * 8  # int16 index columns

    # build per-core arrays
    idx16 = np.zeros((NCORES, 128, COLS), np.int16)
    dstslot = np.full((NCORES, 128, TOTC), 127.0, np.float32)
    dstrow = np.full((NCORES, NTILES, CMAX * P), 127.0, np.float32)

    for c in range(NCORES):
        col = 0
        for t in range(NTILES):
            for (lst, nchunk) in ((low_lists[c][t], CL[t]), (high_lists[c][t], CH[t])):
                idx, dl = lst
                nslot = nchunk * P
                vi = np.zeros(nslot, np.int16)
                vi[: len(idx)] = idx.astype(np.int16)
                vd = np.full(nslot, 127.0, np.float32)
                vd[: len(dl)] = dl.astype(np.float32)
                if nchunk > 0:
                    # idx16 wrapped layout: slot i -> [i%16, col + i//16]
                    seg = vi.reshape(-1, 16).T  # [16, nslot/16]
                    for rep in range(8):
                        idx16[c, rep * 16 : rep * 16 + 16, col : col + nslot // 16] = seg
                    # dstslot: [e, chunkcol] = dl of slot chunk*128+e
                    cbase = np.searchsorted(CUM, 0)  # placeholder
                    col += nslot // 16
                # record dstslot/dstrow below using tile-relative positions
            # fill dstslot/dstrow for this tile
            cbase = CUM[t]
            nslot_t = C[t] * P
            vd_all = np.full(nslot_t, 127.0, np.float32)
            lo_idx, lo_dl = low_lists[c][t]
            hi_idx, hi_dl = high_lists[c][t]
            vd_all[: len(lo_dl)] = lo_dl
            hbase = CL[t] * P
            vd_all[hbase : hbase + len(hi_dl)] = hi_dl
            dstslot[c, :, cbase : cbase + C[t]] = vd_all.reshape(C[t], P).T
            dstrow[c, t, : nslot_t] = vd_all

    # gather segment offsets (in idx16 columns), per tile: (lo_off, hi_off)
    seg_off = []
    col = 0
    for t in range(NTILES):
        lo = col
        col += CL[t] * 8
        hi = col
        col += CH[t] * 8
        seg_off.append((lo, hi))
    assert col == COLS

    # packed one-hot matrices for the bf16 path: Pm[e, c*128+d], PT[d, c*128+e]
    import ml_dtypes as _mld2
    pm_u8 = np.zeros((NCORES, NTILES, 128, CMAX * P), _mld2.bfloat16)
    pt_u8 = np.zeros((NCORES, NTILES, 128, CMAX * P), _mld2.bfloat16)
    rng = np.arange(P, dtype=np.float32)
    for c in range(NCORES):
        for t in range(NTILES):
            nslot_t = C[t] * P
            oh = (dstrow[c, t, :nslot_t, None] == rng[None, :]).astype(_mld2.bfloat16)
            for cc in range(C[t]):
                blk = oh[cc * P : (cc + 1) * P, :]  # [e, d]
                pm_u8[c, t, :, cc * P : (cc + 1) * P] = blk
                pt_u8[c, t, :, cc * P : (cc + 1) * P] = blk.T

    pl = Plan()
    pl.pm_u8, pl.pt_u8 = pm_u8, pt_u8
    pl.src, pl.dst = src, dst
    pl.CL, pl.CH, pl.C, pl.CMAX, pl.CUM = CL, CH, C, CMAX, CUM
    pl.TOTC, pl.COLS, pl.seg_off = TOTC, COLS, seg_off
    pl.idx16 = idx16
    pl.dstslot = dstslot.astype(NPDT)
    import ml_dtypes as _mld
    pl.dstrow = dstrow.astype(_mld.bfloat16)
    return pl


# ------------------------------------------------------------ device builds
def build_d1(elem_out):
    """dense: out[6272, 264] = xT_shard.T @ [W | Wa] + [b | 0]"""
    nc = bacc.Bacc("TRN2", target_bir_lowering=False, debug=False, num_devices=NCORES)
    xT = nc.dram_tensor("xT", [FIN, SHARD], DT, kind="ExternalInput")
    wcat = nc.dram_tensor("wcat", [P, 2 * elem_out], DT, kind="ExternalInput")
    brep = nc.dram_tensor("brep", [P, elem_out], F32, kind="ExternalInput")
    out = nc.dram_tensor("out", [SHARD, elem_out], F32, kind="ExternalOutput")

    with tile.TileContext(nc) as tc:
        with (
            tc.tile_pool(name="consts", bufs=1) as cpool,
            tc.tile_pool(name="lhs", bufs=3) as lpool,
            tc.tile_pool(name="res", bufs=3) as rpool,
            tc.tile_pool(name="ps", bufs=2, space="PSUM") as ppool,
        ):
            w_sb = cpool.tile([P, 2 * elem_out], DT)
            nc.sync.dma_start(w_sb, wcat.ap())
            b_sb = cpool.tile([P, elem_out], F32)
            nc.sync.dma_start(b_sb, brep.ap())
            for t in range(NTILES):
                xt0 = lpool.tile([P, P], DT, tag="xt0")
                nc.sync.dma_start(xt0, xT.ap()[0:128, t * P : (t + 1) * P])
                xt1 = lpool.tile([P, P], DT, tag="xt1")
                nc.sync.dma_start(xt1, xT.ap()[128:256, t * P : (t + 1) * P])
                ps = ppool.tile([P, elem_out], F32, space="PSUM")
                nc.tensor.matmul(ps, lhsT=xt0, rhs=w_sb[:, 0:elem_out], start=True, stop=False)
                nc.tensor.matmul(ps, lhsT=xt1, rhs=w_sb[:, elem_out:], start=False, stop=True)
                res = rpool.tile([P, elem_out], F32)
                nc.vector.tensor_tensor(out=res, in0=ps, in1=b_sb, op=mybir.AluOpType.add)
                nc.sync.dma_start(out.ap()[t * P : (t + 1) * P, :], res)
    nc.compile()
    return nc


def build_agg(pl, layer):
    """Aggregation dispatch. layer=1: gather T1, produce T2 rows (h2|a2).
    layer=2: gather T2, produce classifier logits [8, 6272]."""
    if layer == 1:
        ELEM, ASOFF, ROWF = ELEM1, AS1_F32OFF, ROWF1
        DFEAT, NH, CH_ = D1, H1, C1  # 256, 4, 64
        ELEM_OUT2 = D2 + 8  # 136 dense-2 output row
    else:
        ELEM, ASOFF, ROWF = ELEM2, AS2_F32OFF, ROWF2
        DFEAT, NH, CH_ = D2, H2, C2  # 128, 4, 32

    NFH = DFEAT // P  # feature partition-tiles (2 for L1, 1 for L2)
    RHS_W = DFEAT + 4  # matmul rhs width: feats + p
    USE_R = not USE_BF16  # fp32r on the big feat matmul
    if USE_R:
        RHS_W = max(RHS_W, 256)  # fp32r needs out free >= 256 for 1cyc/row
    GPAD = 0

    nc = bacc.Bacc("TRN2", target_bir_lowering=False, debug=False, num_devices=NCORES,
                   dynamic_dma_scratch_size=DDS)
    T = nc.dram_tensor("T", [TROWS, ELEM], DT, kind="ExternalInput")
    idx_d = nc.dram_tensor("idx", [128, pl.COLS], mybir.dt.int16, kind="ExternalInput")
    dstslot_d = nc.dram_tensor("dstslot", [128, pl.TOTC], DT, kind="ExternalInput")
    dstrow_d = nc.dram_tensor("dstrow", [NTILES, pl.CMAX * P], mybir.dt.bfloat16, kind="ExternalInput")
    ad_d = nc.dram_tensor("ad", [P, NTILES * 4], F32, kind="ExternalInput")
    if USE_BF16:
        pm_d = nc.dram_tensor("pm8", [NTILES, P, pl.CMAX * P], DT, kind="ExternalInput")
        pt_d = nc.dram_tensor("pt8", [NTILES, P, pl.CMAX * P], DT, kind="ExternalInput")
    iota_row_d = nc.dram_tensor("iota_row", [P, P], DT, kind="ExternalInput")
    iota_col_d = nc.dram_tensor("iota_col", [P, 1], F32, kind="ExternalInput")
    ones_d = nc.dram_tensor("ones1", [1, P], mybir.dt.bfloat16, kind="ExternalInput")
    if layer == 1:
        W2W = ELEM_OUT2 if USE_BF16 else 256
        w2cat_d = nc.dram_tensor("w2cat", [P, 2 * W2W], DT, kind="ExternalInput")
        b2rep_d = nc.dram_tensor("b2rep", [P, ELEM_OUT2], F32, kind="ExternalInput")
        ident_d = nc.dram_tensor("ident", [P, P], DT, kind="ExternalInput")
        out = nc.dram_tensor("out", [SHARD, ELEM_OUT2], F32, kind="ExternalOutput")
    else:
        wl_d = nc.dram_tensor("wl", [P, 8], DT, kind="ExternalInput")
        bl_d = nc.dram_tensor("bl", [8, 1], F32, kind="ExternalInput")
        ident_d = nc.dram_tensor("ident", [P, P], DT, kind="ExternalInput")
        out = nc.dram_tensor("out", [8, SHARD], F32, kind="ExternalOutput")

    T_lo = T.ap()[0:HIGH_BASE, :]
    T_hi = T.ap()[HIGH_BASE:TROWS, :]

    with tile.TileContext(nc) as tc:
        with (
            tc.tile_pool(name="consts", bufs=1) as cpool,
            tc.tile_pool(name="gather", bufs=HOTBUFS) as gpool,
            tc.tile_pool(name="onehot", bufs=HOTBUFS) as opool,
            tc.tile_pool(name="scores", bufs=HOTBUFS) as spool,
            tc.tile_pool(name="small", bufs=3) as smpool,
            tc.tile_pool(name="drow", bufs=3) as drpool,
            tc.tile_pool(name="psA", bufs=2, space="PSUM") as psA,   # dstrow bcast
            tc.tile_pool(name="psB", bufs=2, space="PSUM") as psB,   # ad scores
            tc.tile_pool(name="psC", bufs=2, space="PSUM") as psC,   # feat accum
            tc.tile_pool(name="psD", bufs=1, space="PSUM") as psD,   # transpose
            tc.tile_pool(name="psE", bufs=1, space="PSUM") as psE,   # dense2/cls
        ):
            # ---- constants / global loads
            idx_sb = cpool.tile([128, pl.COLS], mybir.dt.int16)
            nc.sync.dma_start(idx_sb, idx_d.ap())
            dstslot_sb = cpool.tile([128, pl.TOTC], DT)
            nc.sync.dma_start(dstslot_sb, dstslot_d.ap())
            ad_sb = cpool.tile([P, NTILES * 4], F32)
            nc.sync.dma_start(ad_sb, ad_d.ap())
            if USE_BF16:
                ad_bf = cpool.tile([P, NTILES * 4], mybir.dt.bfloat16)
                nc.vector.tensor_copy(out=ad_bf, in_=ad_sb)
                ad_use = ad_bf
            else:
                ad_use = ad_sb
            iota_row = cpool.tile([P, P], DT)
            nc.sync.dma_start(iota_row, iota_row_d.ap())
            iota_col = cpool.tile([P, 1], F32)
            nc.sync.dma_start(iota_col, iota_col_d.ap())
            ones1 = cpool.tile([1, P], mybir.dt.bfloat16)
            nc.sync.dma_start(ones1, ones_d.ap())
            ident = cpool.tile([P, P], DT)
            nc.sync.dma_start(ident, ident_d.ap())
            if layer == 1:
                w2_sb = cpool.tile([P, 2 * W2W], DT)
                nc.sync.dma_start(w2_sb, w2cat_d.ap())
                if not USE_BF16:
                    w2_r = cpool.tile([P, 2 * W2W], F32)
                    nc.vector.tensor_copy(out=w2_r.bitcast(F32R), in_=w2_sb)
                else:
                    w2_r = w2_sb
                b2_sb = cpool.tile([P, ELEM_OUT2], F32)
                nc.sync.dma_start(b2_sb, b2rep_d.ap())
            else:
                wl_sb = cpool.tile([P, 8], DT)
                nc.sync.dma_start(wl_sb, wl_d.ap())
                bl_sb = cpool.tile([8, 1], F32)
                nc.sync.dma_start(bl_sb, bl_d.ap())
                outbuf = cpool.tile([8, SHARD], F32)

            for t in range(NTILES):
                C = pl.C[t]
                CL, CH = pl.CL[t], pl.CH[t]
                lo_off, hi_off = pl.seg_off[t]

                # ---- gather rows for this tile's edge slots
                G = gpool.tile([128, pl.CMAX * ELEM + GPAD], DT, tag="G", name="G")[:, : C * ELEM]
                G3 = G.rearrange("p (c e) -> p c e", e=ELEM)
                for (nch, cb, off, src_ap) in (
                    (CL, 0, lo_off, T_lo),
                    (CH, CL, hi_off, T_hi),
                ):
                    for p0 in range(0, nch, MAXG):
                        pc = min(MAXG, nch - p0)
                        nc.gpsimd.dma_gather(
                            G3[:, cb + p0 : cb + p0 + pc, :],
                            src_ap,
                            idx_sb[:, off + p0 * 8 : off + (p0 + pc) * 8],
                            pc * P,
                            pc * P,
                            ELEM,
                        )

                if USE_BF16:
                    PT = opool.tile([128, pl.CMAX * P], DT, tag="PT", name="PT")[:, : C * P]
                    nc.sync.dma_start(PT, pt_d.ap()[t, :, 0 : C * P])
                    Pm = opool.tile([128, pl.CMAX * P], DT, tag="Pm", name="Pm")[:, : C * P]
                    nc.sync.dma_start(Pm, pm_d.ap()[t, :, 0 : C * P])
                else:
                    # ---- PT one-hot [d, C*128]: PE broadcast of dstrow + is_equal
                    drow = drpool.tile([1, pl.CMAX * P], mybir.dt.bfloat16, tag="drow", name="drow")[:, : C * P]
                    nc.sync.dma_start(drow, dstrow_d.ap()[t : t + 1, 0 : C * P])
                    PT = opool.tile([128, pl.CMAX * P], F32, tag="PT", name="PT")[:, : C * P]
                    for s0 in range(0, C * P, 512):
                        seg = min(512, C * P - s0)
                        psd = psA.tile([P, 512], F32, tag="psd", space="PSUM")
                        nc.tensor.matmul(
                            psd[:, 0:seg], lhsT=ones1, rhs=drow[:, s0 : s0 + seg],
                            start=True, stop=True,
                        )
                        dsb = drpool.tile([P, 512], F32, tag="dsb", name="dsb")
                        nc.scalar.copy(out=dsb[:, 0:seg], in_=psd[:, 0:seg])
                        nc.vector.tensor_scalar(
                            out=PT[:, s0 : s0 + seg], in0=dsb[:, 0:seg],
                            scalar1=iota_col, scalar2=None,
                            op0=mybir.AluOpType.is_equal,
                        )

                    # ---- P one-hot [e, C*128] (DT)
                    Pm = opool.tile([128, pl.CMAX * P], DT, tag="Pm", name="Pm")[:, : C * P]
                    if USE_R:
                        Pm = Pm.bitcast(F32R)
                    Pm3 = Pm.rearrange("p (c d) -> p c d", d=P)
                    nc.vector.tensor_tensor(
                        out=Pm3,
                        in0=dstslot_sb[:, pl.CUM[t] : pl.CUM[t] + C].to_broadcast([128, C, P]),
                        in1=iota_row.unsqueeze(1).to_broadcast([128, C, P]),
                        op=mybir.AluOpType.is_equal,
                    )

                # ---- a_d per edge: psum_sc[e, c*4+h] = sum_d PT[d, e] * a_d[d, h]
                pssc = psB.tile([P, pl.CMAX * 4], F32, tag="pssc", space="PSUM")
                for c in range(C):
                    nc.tensor.matmul(
                        pssc[:, c * 4 : (c + 1) * 4],
                        lhsT=PT[:, c * P : (c + 1) * P],
                        rhs=ad_use[:, t * 4 : (t + 1) * 4],
                        start=True, stop=True,
                    )

                # ---- scores: e = leaky(a_s + a_d); p = exp(e)
                G_f = G.bitcast(F32)
                G_f3 = G_f.rearrange("p (c e) -> p c e", e=ROWF)
                as_view = G_f3[:, :, ASOFF : ASOFF + 4]
                esum = spool.tile([128, pl.CMAX * 4], F32, tag="esum", name="esum")[:, : C * 4]
                esum3 = esum.rearrange("p (c h) -> p c h", h=4)
                nc.vector.tensor_tensor(
                    out=esum3, in0=as_view,
                    in1=pssc[:, : C * 4].rearrange("p (c h) -> p c h", h=4),
                    op=mybir.AluOpType.add,
                )
                nc.vector.scalar_tensor_tensor(
                    out=esum, in0=esum, scalar=NEG_ATT, in1=esum,
                    op0=mybir.AluOpType.mult, op1=mybir.AluOpType.max,
                )
                # p -> written into the a_s slots (consumed above) to form
                # a contiguous matmul rhs [feats | p] per chunk
                RA = spool.tile([128, pl.CMAX * RHS_W], DT, tag="RA", name="RA")[:, : C * RHS_W]
                if USE_R:
                    RA = RA.bitcast(F32R)
                RA3 = RA.rearrange("p (c e) -> p c e", e=RHS_W)
                feat4 = G3[:, :, 0:DFEAT].rearrange("p c (h f) -> p c h f", f=CH_)
                feat4o = RA3[:, :, 0:DFEAT].rearrange("p c (h f) -> p c h f", f=CH_)
                esum4 = esum.rearrange("p (c h) -> p c h", h=4)
                if USE_BF16:
                    # exp pre-expanded by ACT (redundant transcendentals are
                    # cheaper than a DVE slow-mode broadcast multiply): the
                    # all-contiguous bf16 multiply then runs in DVE 2x mode.
                    pexp = spool.tile([128, pl.CMAX * DFEAT], DT, tag="pexp", name="pexp")[:, : C * DFEAT]
                    pexp4 = pexp.rearrange("p (c h f) -> p c h f", h=4, f=CH_)
                    nc.scalar.activation(
                        out=pexp4,
                        in_=esum4.to_broadcast([128, C, 4, CH_]),
                        func=mybir.ActivationFunctionType.Exp,
                    )
                    nc.vector.tensor_copy(
                        out=RA3[:, :, DFEAT : DFEAT + 4], in_=pexp4[:, :, :, 0:1].rearrange("p c h f -> p c (h f)")
                    )
                    nc.vector.tensor_tensor(out=feat4o, in0=feat4, in1=pexp4, op=mybir.AluOpType.mult)
                else:
                    p_sb = spool.tile([128, pl.CMAX * 4], F32, tag="p_sb", name="p_sb")[:, : C * 4]
                    p_sb3 = p_sb.rearrange("p (c h) -> p c h", h=4)
                    nc.scalar.activation(
                        out=p_sb3,
                        in_=esum4,
                        func=mybir.ActivationFunctionType.Exp,
                    )
                    # p into the rhs tail columns (cast/round on write)
                    nc.vector.tensor_copy(out=RA3[:, :, DFEAT : DFEAT + 4], in_=p_sb3)
                    pb = p_sb3.to_broadcast([128, C, 4, CH_])
                    nc.vector.tensor_tensor(out=feat4o, in0=feat4, in1=pb, op=mybir.AluOpType.mult)

                # ---- accumulate: out[d, 0:DFEAT]=feats, [DFEAT:DFEAT+4]=denom
                psout = psC.tile([P, RHS_W], F32, tag="psout", space="PSUM")
                for c in range(C):
                    nc.tensor.matmul(
                        psout,
                        lhsT=Pm[:, c * P : (c + 1) * P],
                        rhs=RA[:, c * RHS_W : (c + 1) * RHS_W],
                        start=(c == 0), stop=(c == C - 1),
                    )

                # ---- normalize
                recip = smpool.tile([P, 4], F32, tag="recip")
                nc.vector.reciprocal(recip, psout[:, DFEAT : DFEAT + 4])
                o1 = smpool.tile([P, DFEAT], F32, tag="o1")
                nc.vector.tensor_tensor(
                    out=o1.rearrange("p (h f) -> p h f", f=CH_),
                    in0=psout[:, 0:DFEAT].rearrange("p (h f) -> p h f", f=CH_),
                    in1=recip.to_broadcast([P, 4, CH_]),
                    op=mybir.AluOpType.mult,
                )

                if layer == 1:
                    # relu -> r1 (DT), transpose, dense-2, +b2, write T2 rows
                    r1 = smpool.tile([P, DFEAT], DT, tag="r1")
                    nc.vector.tensor_scalar(
                        out=r1, in0=o1, scalar1=0.0, scalar2=None,
                        op0=mybir.AluOpType.max,
                    )
                    pse = psE.tile([P, W2W], F32, tag="pse", space="PSUM")
                    for h in range(NFH):
                        pst = psD.tile([P, P], DT, tag="pst", space="PSUM")
                        nc.tensor.transpose(pst, r1[:, h * P : (h + 1) * P], ident)
                        r1T = smpool.tile([P, P], DT, tag="r1T")
                        r1To = r1T.bitcast(F32R) if not USE_BF16 else r1T
                        nc.scalar.copy(out=r1To, in_=pst)
                        nc.tensor.matmul(
                            pse,
                            lhsT=r1To,
                            rhs=w2_r.bitcast(F32R)[:, h * W2W : h * W2W + W2W]
                            if not USE_BF16 else w2_r[:, h * W2W : h * W2W + W2W],
                            start=(h == 0), stop=(h == NFH - 1),
                        )
                    t2row = smpool.tile([P, ELEM_OUT2], F32, tag="t2row")
                    nc.vector.tensor_tensor(out=t2row, in0=pse[:, 0:ELEM_OUT2], in1=b2_sb, op=mybir.AluOpType.add)
                    nc.sync.dma_start(out.ap()[t * P : (t + 1) * P, :], t2row)
                else:
                    # leaky(0.01) -> transpose -> classifier -> outbuf
                    o2 = smpool.tile([P, DFEAT], DT, tag="o2")
                    nc.vector.scalar_tensor_tensor(
                        out=o2, in0=o1, scalar=NEG_ACT, in1=o1,
                        op0=mybir.AluOpType.mult, op1=mybir.AluOpType.max,
                    )
                    pst = psD.tile([P, P], DT, tag="pst", space="PSUM")
                    nc.tensor.transpose(pst, o2, ident)
                    o2T = smpool.tile([P, P], DT, tag="o2T")
                    nc.scalar.copy(out=o2T, in_=pst)
                    psc = psE.tile([8, P], F32, tag="psc", space="PSUM")
                    nc.tensor.matmul(psc, lhsT=wl_sb, rhs=o2T, start=True, stop=True)
                    nc.vector.tensor_scalar(
                        out=outbuf[:, t * P : (t + 1) * P], in0=psc,
                        scalar1=bl_sb, scalar2=None, op0=mybir.AluOpType.add,
                    )
            if layer == 2:
                nc.sync.dma_start(out.ap(), outbuf)
    nc.compile()
    return nc


# ------------------------------------------------------------------ helpers
def _wcat(W, att_src, att_dst, heads, chan):
    """[W | W@blockdiag(att_src) | W@blockdiag(att_dst)] -> [K, D+8]"""
    K, Dh = W.shape
    wa_s = np.zeros((K, heads), np.float32)
    wa_d = np.zeros((K, heads), np.float32)
    for h in range(heads):
        wa_s[:, h] = W[:, h * chan : (h + 1) * chan] @ att_src[h]
        wa_d[:, h] = W[:, h * chan : (h + 1) * chan] @ att_dst[h]
    return np.concatenate([W, wa_s, wa_d], axis=1).astype(np.float32)


def _chunk_major(Wfull, width=None):
    """[256, E] -> [128, 2*width] (K-chunk-major for SBUF, zero-padded)"""
    e = Wfull.shape[1]
    width = width or e
    out = np.zeros((128, 2 * width), Wfull.dtype)
    out[:, 0:e] = Wfull[0:128, :]
    out[:, width : width + e] = Wfull[128:256, :]
    return out


def _make_table(h_plus_b, a_s, rowf, elem, asoff):
    """Assemble gather table [TROWS, elem] in DT with a_s stored as f32."""
    dfeat = h_plus_b.shape[1]
    Tf = np.zeros((TROWS, rowf), np.float32)
    if USE_BF16:
        Tb = np.zeros((TROWS, elem), NPDT)
        rows = row_of_node(np.arange(N))
        Tb[rows, 0:dfeat] = h_plus_b.astype(NPDT)
        Tf_view = Tb.view(np.uint8).reshape(TROWS, elem * 2)
        asf = np.zeros((TROWS, 4), np.float32)
        asf[rows] = a_s
        asf[0] = DUMMY_AS
        asf[HIGH_BASE] = DUMMY_AS
        Tf_view[:, asoff * 4 : asoff * 4 + 16] = asf.view(np.uint8).reshape(TROWS, 16)
        return Tb
    else:
        rows = row_of_node(np.arange(N))
        Tf[rows, 0:dfeat] = h_plus_b
        Tf[:, asoff : asoff + 4] = DUMMY_AS
        Tf[rows, asoff : asoff + 4] = a_s
        return Tf


def _ad_input(a_d):
    """[NPAD, 4] padded a_d -> per-core [128, NTILES*4]"""
    out = np.zeros((NCORES, P, NTILES * 4), np.float32)
    for c in range(NCORES):
        blk = a_d[c * SHARD : (c + 1) * SHARD].reshape(NTILES, P, 4)
        out[c] = blk.transpose(1, 0, 2).reshape(P, NTILES * 4)
    return out


_CACHE = {}


def _run(nc, in_maps, tag):
    trace = TRACE
    if trace:
        try:
            from antenv.axon_hooks import get_axon_ntff_profile_hook  # noqa: F401
        except ImportError:
            trace = False
    res = run_bass_kernel_spmd(nc, in_maps, core_ids=list(range(NCORES)), trace=trace)
    if trace and res.exec_time_ns:
        print(f"[{tag}] exec_time_ns = {res.exec_time_ns}", file=sys.stderr)
        _CACHE.setdefault("times", {})[tag] = res.exec_time_ns
    return res.results


# -------------------------------------------------------------------- main
def kernel(x, edge_index, W1, att_src1, att_dst1, b1, W2, att_src2, att_dst2, b2, Wl, bl):
    x = np.asarray(x, np.float32)
    W1 = np.asarray(W1, np.float32)
    W2 = np.asarray(W2, np.float32)
    Wl = np.asarray(Wl, np.float32)
    b1 = np.asarray(b1, np.float32)
    b2 = np.asarray(b2, np.float32)
    bl = np.asarray(bl, np.float32)
    att_src1 = np.asarray(att_src1, np.float32)
    att_dst1 = np.asarray(att_dst1, np.float32)
    att_src2 = np.asarray(att_src2, np.float32)
    att_dst2 = np.asarray(att_dst2, np.float32)

    pl = build_plan(np.asarray(edge_index))

    iota_row = np.tile(np.arange(P, dtype=np.float32)[None, :], (P, 1)).astype(NPDT)
    iota_col = np.arange(P, dtype=np.float32).reshape(P, 1)
    import ml_dtypes as _mld
    ones1 = np.ones((1, P), _mld.bfloat16)
    ident = np.eye(P, dtype=np.float32).astype(NPDT)

    # ---------------- D1: dense layer-1
    w1cat = _wcat(W1, att_src1, att_dst1, H1, C1)  # [256, 264]
    ELEM_D1 = D1 + 8
    xT = np.zeros((FIN, NPAD), np.float32)
    xT[:, 0:N] = x.T
    d1_in = []
    for c in range(NCORES):
        d1_in.append({
            "xT": xT[:, c * SHARD : (c + 1) * SHARD].astype(NPDT),
            "wcat": _chunk_major(w1cat).astype(NPDT),
            "brep": np.tile(np.concatenate([b1, np.zeros(8, np.float32)])[None, :], (P, 1)),
        })
    if "d1" not in _CACHE:
        _CACHE["d1"] = build_d1(ELEM_D1)
    r1 = _run(_CACHE["d1"], d1_in, "d1")
    ha1 = np.concatenate([r["out"] for r in r1], axis=0)[0:N]  # [N, 264] = [h1+b1 | a_s | a_d]

    # ---------------- host: assemble T1 + a_d input
    T1 = _make_table(ha1[:, 0:D1], ha1[:, D1 : D1 + 4], ROWF1, ELEM1, AS1_F32OFF)
    ad1 = np.zeros((NPAD, 4), np.float32)
    ad1[0:N] = ha1[:, D1 + 4 : D1 + 8]
    ad1_in = _ad_input(ad1)

    # ---------------- D2: layer-1 aggregation + dense layer-2
    w2cat = _wcat(W2, att_src2, att_dst2, H2, C2)  # [256, 136]
    ELEM_OUT2 = D2 + 8
    d2_in = []
    for c in range(NCORES):
        d2_in.append({
            "T": T1,
            "idx": pl.idx16[c],
            "dstslot": pl.dstslot[c],
            "dstrow": pl.dstrow[c],
            "ad": ad1_in[c],
            **({"pm8": pl.pm_u8[c], "pt8": pl.pt_u8[c]} if USE_BF16 else {}),
            "iota_row": iota_row,
            "iota_col": iota_col,
            "ones1": ones1,
            "ident": ident,
            "w2cat": _chunk_major(w2cat, ELEM_OUT2 if USE_BF16 else 256).astype(NPDT),
            "b2rep": np.tile(np.concatenate([b2, np.zeros(8, np.float32)])[None, :], (P, 1)),
        })
    key = ("d2", pl.COLS, pl.TOTC, tuple(pl.C))
    if key not in _CACHE:
        _CACHE[key] = build_agg(pl, 1)
    r2 = _run(_CACHE[key], d2_in, "d2")
    ha2 = np.concatenate([r["out"] for r in r2], axis=0)[0:NPAD]  # [NPAD, 136]
    ha2n = np.zeros((N, ELEM_OUT2), np.float32)
    ha2n[:, :] = ha2[0:N]

    # ---------------- host: assemble T2 + a_d input
    T2 = _make_table(ha2n[:, 0:D2], ha2n[:, D2 : D2 + 4], ROWF2, ELEM2, AS2_F32OFF)
    ad2 = np.zeros((NPAD, 4), np.float32)
    ad2[0:N] = ha2n[:, D2 + 4 : D2 + 8]
    ad2_in = _ad_input(ad2)

    # ---------------- D3: layer-2 aggregation + classifier
    wl8 = np.zeros((P, 8), np.float32)
    wl8[:, 0:NCLS] = Wl
    bl8 = np.zeros((8, 1), np.float32)
    bl8[0:NCLS, 0] = bl
    d3_in = []
    for c in range(NCORES):
        d3_in.append({
            "T": T2,
            "idx": pl.idx16[c],
            "dstslot": pl.dstslot[c],
            "dstrow": pl.dstrow[c],
            "ad": ad2_in[c],
            **({"pm8": pl.pm_u8[c], "pt8": pl.pt_u8[c]} if USE_BF16 else {}),
            "iota_row": iota_row,
            "iota_col": iota_col,
            "ones1": ones1,
            "ident": ident,
            "wl": wl8.astype(NPDT),
            "bl": bl8,
        })
    key3 = ("d3", pl.COLS, pl.TOTC, tuple(pl.C))
    if key3 not in _CACHE:
        _CACHE[key3] = build_agg(pl, 2)
    r3 = _run(_CACHE[key3], d3_in, "d3")

    out = np.zeros((N, NCLS), np.float32)
    for c in range(NCORES):
        blk = r3[c]["out"]  # [8, SHARD]
        g0, g1 = c * SHARD, min((c + 1) * SHARD, N)
        if g1 > g0:
            out[g0:g1] = blk[0:NCLS, 0 : g1 - g0].T
    return out
